# revision 9
# baseline (speedup 1.0000x reference)
"""Trainium2 Bass kernel for nn_CrossAttentionBlock.

Math: with key/value seq_len == 1 the attention softmax is identically 1, so
q/k (and masked_x entirely) never affect the output:

    out[n, :] = LN(((graph_vec @ Wv.T + bv) @ Wiv.T + biv) @ Wout.T + bout)[batch_indices[n]]

i.e. a 128-row lookup table indexed by batch_indices. Strategy per core
(data-parallel over nodes, 8 cores x 50000 nodes):

  1. prologue: compute the [128, 128] table on-device from host-transposed
     weight layouts (3 matmuls + bias rank-1 updates + bn_stats LayerNorm),
     cast to fp16 (rel err ~5e-4, far under the 2e-2 gate).
  2. main loop per 1024-node superblock (2 PSUM banks):
       - broadcast idx (fp16, exact for ints<2048) across partitions:
         2x PE K=1 matmul -> PSUM, or GpSimd partition_broadcast -> SBUF
       - is_equal against a partition-iota column -> one-hot^T (fp16)
         on DVE (PE superblocks) or GpSimd (pool superblocks)
       - 8x PE matmuls: out[node, h] = onehotT.T @ tbl   (single fp16 pass)
       - one PSUM -> SBUF staging copy (Scalar, some DVE)
       - one 512 KiB DMA store per superblock

The only irreducible HBM traffic is the 25.7 MiB/core fp32 output write
(~75 us at ~350 GB/s); every other engine is scheduled to stay under that.
idx is loaded partition-parallel as [49, 1024] (a [1, N] row load would
serialize ~38 us on one SBUF partition's write port).
"""

import sys

if "/opt/trn_rl_repo" not in sys.path:
    sys.path.insert(0, "/opt/trn_rl_repo")

import numpy as np

import concourse.bass as bass
import concourse.bacc as bacc
import concourse.tile as tile
from concourse import mybir
from concourse import bass_utils

F32 = mybir.dt.float32
F16 = mybir.dt.float16

N_NODES = 400000
H = 128          # hidden
G = 256          # graph_dim
B = 128          # batch (table rows)
N_CORES = 8
NSHARD = N_NODES // N_CORES          # 50000
SB = 1024                            # nodes per superblock (2 PSUM banks)
NPAD = 50176                         # 49 * 1024, per-core padded shard
NSB = NPAD // SB                     # 49
EPS = 1e-5

# Schedule knobs (variant tuple: (pool_mod, copy_sched))
#   pool_mod: m -> superblocks with sb % m == m-1 broadcast+eq on GpSimd
#             (None = all broadcasts on PE, all eq on DVE)
#   copy_sched: string over {"A","V"} cycled per superblock for the
#               PSUM->stage copy engine (Scalar/DVE; GpSimd cannot read PSUM)
DEFAULT_VARIANT = (3, "AAAAAAAV")


def _row1(ap):
    """View a 1-D DRAM AP as [1, N]."""
    return bass.AP(tensor=ap.tensor, offset=ap.offset, ap=[[0, 1]] + list(ap.ap))


def _bcast128(ap):
    """View a 1-D DRAM AP as [128, N] replicated across partitions."""
    return bass.AP(tensor=ap.tensor, offset=ap.offset, ap=[[0, 128]] + list(ap.ap))


def build_bass(variant=DEFAULT_VARIANT):
    pool_mod, copy_sched = variant
    nc = bacc.Bacc("TRN2", target_bir_lowering=False)

    # host-transposed layouts (see _prep_in_maps)
    gvt_d = nc.dram_tensor("gvT", [G, B], F32, kind="ExternalInput")
    wvt_d = nc.dram_tensor("WvT", [G, H], F32, kind="ExternalInput")
    wivt_d = nc.dram_tensor("WivT", [H, H], F32, kind="ExternalInput")
    woutt_d = nc.dram_tensor("WoutT", [H, H], F32, kind="ExternalInput")
    bv_d = nc.dram_tensor("bv", [H], F32, kind="ExternalInput")
    biv_d = nc.dram_tensor("biv", [H], F32, kind="ExternalInput")
    bout_d = nc.dram_tensor("bout", [H], F32, kind="ExternalInput")
    gamma_d = nc.dram_tensor("gamma", [H], F32, kind="ExternalInput")
    beta_d = nc.dram_tensor("beta", [H], F32, kind="ExternalInput")
    idx_d = nc.dram_tensor("idx", [NSB, SB], F16, kind="ExternalInput")
    out_d = nc.dram_tensor("out", [NPAD, H], F32, kind="ExternalOutput")

    with tile.TileContext(nc) as tc:
        with (
            tc.tile_pool(name="singles", bufs=1) as singles,
            tc.tile_pool(name="oh", bufs=3) as oh_pool,
            tc.tile_pool(name="idxc", bufs=6) as idxc_pool,
            tc.tile_pool(name="bcsb", bufs=2) as bcsb_pool,
            tc.tile_pool(name="bps", bufs=2, space="PSUM") as bc_ps_pool,
            tc.tile_pool(name="ops", bufs=2, space="PSUM") as out_ps_pool,
            tc.tile_pool(name="stage", bufs=3) as stage_pool,
        ):
            # ---------- constants & weights ----------
            # gvT [256, 128] exceeds 128 partitions; load as two [128, 128]
            gvt0 = singles.tile([128, B], F32, tag="gvt0")
            nc.sync.dma_start(out=gvt0, in_=gvt_d[0:128, :])
            gvt1 = singles.tile([128, B], F32, tag="gvt1")
            nc.sync.dma_start(out=gvt1, in_=gvt_d[128:256, :])
            wvt0 = singles.tile([128, H], F32, tag="wvt0")
            nc.sync.dma_start(out=wvt0, in_=wvt_d[0:128, :])
            wvt1 = singles.tile([128, H], F32, tag="wvt1")
            nc.sync.dma_start(out=wvt1, in_=wvt_d[128:256, :])
            wivt_sb = singles.tile([H, H], F32, tag="wivt")
            nc.sync.dma_start(out=wivt_sb, in_=wivt_d[:, :])
            woutt_sb = singles.tile([H, H], F32, tag="woutt")
            nc.sync.dma_start(out=woutt_sb, in_=woutt_d[:, :])

            bv_sb = singles.tile([1, H], F32, tag="bv")
            nc.sync.dma_start(out=bv_sb, in_=_row1(bv_d[:]))
            biv_sb = singles.tile([1, H], F32, tag="biv")
            nc.sync.dma_start(out=biv_sb, in_=_row1(biv_d[:]))
            bout_sb = singles.tile([1, H], F32, tag="bout")
            nc.sync.dma_start(out=bout_sb, in_=_row1(bout_d[:]))

            gamma_gr = singles.tile([128, H], F32, tag="gamma_gr")
            nc.gpsimd.dma_start(out=gamma_gr, in_=_bcast128(gamma_d[:]))
            beta_gr = singles.tile([128, H], F32, tag="beta_gr")
            nc.gpsimd.dma_start(out=beta_gr, in_=_bcast128(beta_d[:]))

            ones32 = singles.tile([1, 128], F32, tag="ones32")
            nc.vector.memset(ones32, 1.0)
            ones16 = singles.tile([1, 128], F16, tag="ones16")
            nc.vector.memset(ones16, 1.0)
            eps_sb = singles.tile([128, 1], F32, tag="eps")
            nc.vector.memset(eps_sb, EPS)

            iota_i = singles.tile([128, 1], mybir.dt.int32, tag="iota_i")
            nc.gpsimd.iota(iota_i, [[0, 1]], base=0, channel_multiplier=1)
            iota_f = singles.tile([128, 1], F32, tag="iota_f")
            nc.vector.tensor_copy(out=iota_f, in_=iota_i)

            tc.strict_bb_all_engine_barrier()

            # ---------- table prologue ----------
            # PSUM borrowed from the out pool (same tag -> same 2 slots).
            # v.T[h, b] = sum_g WvT[g, h] * gvT[g, b] + bv[h]
            vt_ps = out_ps_pool.tile([128, SB], F32, tag="outps")
            nc.tensor.matmul(vt_ps[:, 0:128], wvt0, gvt0, start=True, stop=False)
            nc.tensor.matmul(vt_ps[:, 0:128], wvt1, gvt1, start=False, stop=False)
            nc.tensor.matmul(vt_ps[:, 0:128], bv_sb, ones32, start=False, stop=True)
            vt_sb = singles.tile([128, 128], F32, tag="vt_sb")
            nc.scalar.copy(out=vt_sb, in_=vt_ps[:, 0:128])

            # v2.T[j, b] = sum_h WivT[h, j] * vT[h, b] + biv[j]
            v2t_ps = out_ps_pool.tile([128, SB], F32, tag="outps")
            nc.tensor.matmul(v2t_ps[:, 0:128], wivt_sb, vt_sb, start=True, stop=False)
            nc.tensor.matmul(v2t_ps[:, 0:128], biv_sb, ones32, start=False, stop=True)
            v2t_sb = singles.tile([128, 128], F32, tag="v2t_sb")
            nc.scalar.copy(out=v2t_sb, in_=v2t_ps[:, 0:128])

            # ao[b, h2] = sum_j v2T[j, b] * WoutT[j, h2] + bout[h2]
            ao_ps = out_ps_pool.tile([128, SB], F32, tag="outps")
            nc.tensor.matmul(ao_ps[:, 0:128], v2t_sb, woutt_sb, start=True, stop=False)
            nc.tensor.matmul(ao_ps[:, 0:128], ones32, bout_sb, start=False, stop=True)

            # LayerNorm over free dim
            stats = singles.tile([128, 6], F32, tag="stats")
            nc.vector.bn_stats(out=stats, in_=ao_ps[:, 0:128])
            mv = singles.tile([128, 2], F32, tag="mv")
            nc.vector.bn_aggr(out=mv, in_=stats)
            rstd = singles.tile([128, 1], F32, tag="rstd")
            nc.scalar.activation(
                rstd, mv[:, 1:2], mybir.ActivationFunctionType.Sqrt,
                bias=eps_sb, scale=1.0,
            )
            nc.vector.reciprocal(out=rstd, in_=rstd)

            tbl = singles.tile([128, 128], F32, tag="tbl")
            nc.vector.tensor_scalar(
                out=tbl, in0=ao_ps[:, 0:128],
                scalar1=mv[:, 0:1], scalar2=rstd,
                op0=mybir.AluOpType.subtract, op1=mybir.AluOpType.mult,
            )
            tbl2 = singles.tile([128, 128], F32, tag="tbl2")
            nc.vector.tensor_mul(out=tbl2, in0=tbl, in1=gamma_gr)
            tbl3 = singles.tile([128, 128], F32, tag="tbl3")
            nc.vector.tensor_add(out=tbl3, in0=tbl2, in1=beta_gr)
            tbl_h = singles.tile([128, 128], F16, tag="tbl_h")
            nc.vector.tensor_copy(out=tbl_h, in_=tbl3)

            # ---------- main gather loop: one superblock = 1024 nodes ----------
            for sb in range(NSB):
                use_pool = pool_mod is not None and sb % pool_mod == pool_mod - 1
                # idx chunk for this superblock: 2 KB onto partition 0,
                # triggered from the cheap Pool queue; ring depth 6 lets the
                # loads run ~5 superblocks ahead of the consumers.
                idxc = idxc_pool.tile([1, SB], F16, tag="idxc")
                nc.gpsimd.dma_start(out=idxc, in_=idx_d[sb:sb + 1, :])
                if use_pool:
                    bc_sb = bcsb_pool.tile([128, SB], F16, tag="bcsb")
                    nc.gpsimd.partition_broadcast(bc_sb, idxc)
                    eq_in = bc_sb
                else:
                    bc_ps = bc_ps_pool.tile([128, SB], F32, tag="bcast")
                    nc.tensor.matmul(
                        bc_ps[:, 0:512], ones16, idxc[:, 0:512],
                        start=True, stop=True,
                    )
                    nc.tensor.matmul(
                        bc_ps[:, 512:SB], ones16, idxc[:, 512:SB],
                        start=True, stop=True,
                    )
                    eq_in = bc_ps
                # onehotT[j, p] = (idx[p] == j)
                oh = oh_pool.tile([128, SB], F16, tag="oh")
                eq_eng = nc.gpsimd if use_pool else nc.vector
                eq_eng.tensor_scalar(
                    out=oh, in0=eq_in,
                    scalar1=iota_f, scalar2=None,
                    op0=mybir.AluOpType.is_equal,
                )
                # out[p, h] = sum_j onehotT[j, p] * table[j, h]
                out_ps = out_ps_pool.tile([128, SB], F32, tag="outps")
                for t in range(SB // 128):
                    sl = slice(t * 128, (t + 1) * 128)
                    nc.tensor.matmul(
                        out_ps[:, sl], oh[:, sl], tbl_h,
                        start=True, stop=True,
                    )
                stage = stage_pool.tile([128, SB], F32, tag="stage")
                if copy_sched[sb % len(copy_sched)] == "A":
                    nc.scalar.copy(out=stage, in_=out_ps)
                else:
                    nc.vector.tensor_copy(out=stage, in_=out_ps)
                # Node order is host-permuted so partition p owns DRAM rows
                # [p*NT, (p+1)*NT): every store descriptor is a contiguous
                # 8*512B run per partition (full DMA line rate).
                ts = SB // 128                        # 8 tiles per store
                dview = out_d[:, :].rearrange("(p t) c -> p t c", p=128)[
                    :, sb * ts:(sb + 1) * ts, :
                ]
                sview = stage.rearrange("p (t c) -> p t c", c=128)
                nc.sync.dma_start(out=dview, in_=sview)

    nc.finalize()
    return nc


_CACHE = {}


def _get_nc(variant=None):
    key = variant or DEFAULT_VARIANT
    if key not in _CACHE:
        _CACHE[key] = build_bass(variant=key)
    return _CACHE[key]


def _prep_in_maps(inputs):
    f32c = lambda x: np.ascontiguousarray(np.asarray(x), dtype=np.float32)
    win = np.asarray(inputs["Win"], dtype=np.float32)
    bin_ = np.asarray(inputs["bin"], dtype=np.float32)
    shared = {
        "gvT": f32c(np.asarray(inputs["graph_vec"], dtype=np.float32).T),
        "WvT": f32c(np.asarray(inputs["Wv"], dtype=np.float32).T),
        "WivT": f32c(win[2 * H:3 * H, :].T),
        "WoutT": f32c(np.asarray(inputs["Wout"], dtype=np.float32).T),
        "bv": f32c(inputs["bv"]),
        "biv": f32c(bin_[2 * H:3 * H]),
        "bout": f32c(inputs["bout"]),
        "gamma": f32c(inputs["gamma"]),
        "beta": f32c(inputs["beta"]),
    }
    bi = np.asarray(inputs["batch_indices"]).astype(np.int64).reshape(N_CORES, NSHARD)
    idx_pad = np.zeros((N_CORES, NPAD), dtype=np.int64)
    idx_pad[:, :NSHARD] = bi
    # Permute so device tile t covers nodes {p*NT + t}: partition p then owns
    # the contiguous output-row block [p*NT, (p+1)*NT) (contiguous DMA runs).
    nt = NPAD // 128
    idx_tr = idx_pad.reshape(N_CORES, 128, nt).transpose(0, 2, 1)  # [c, t, p]
    idx_flat = idx_tr.reshape(N_CORES, NSB, SB)
    idx_f16 = idx_flat.astype(np.float16)  # exact: values < 2048
    return [
        {**shared, "idx": np.ascontiguousarray(idx_f16[c])}
        for c in range(N_CORES)
    ]


def run_sharded(inputs, trace=False, variant=None, **kwargs):
    """Run the SPMD bass kernel on 8 cores; returns (output, BassKernelResults)."""
    kwargs.pop("precision", None)  # legacy knob
    in_maps = _prep_in_maps(inputs)
    nc = _get_nc(variant)
    res = bass_utils.run_bass_kernel_spmd(
        nc, in_maps, core_ids=list(range(N_CORES)), trace=trace, **kwargs
    )
    shards = [r["out"][:NSHARD] for r in res.results]
    out = np.concatenate(shards, axis=0)
    return out, res


def kernel(**inputs) -> np.ndarray:
    out, _ = run_sharded(inputs)
    return out


# revision 10
# speedup vs baseline: 2.4209x; 2.4209x over previous
"""Trainium2 Bass kernel for nn_CrossAttentionBlock.

Math: with key/value seq_len == 1 the attention softmax is identically 1, so
q/k (and masked_x entirely) never affect the output:

    out[n, :] = LN(((graph_vec @ Wv.T + bv) @ Wiv.T + biv) @ Wout.T + bout)[batch_indices[n]]

i.e. a 128-row lookup table indexed by batch_indices. Strategy per core
(data-parallel over nodes, 8 cores x 50000 nodes):

  1. prologue: compute the [128, 128] table on-device from host-transposed
     weight layouts (3 matmuls + bias rank-1 updates + bn_stats LayerNorm),
     then split into an fp8e4m3 hi+lo pair (rel err ~1e-3, gate is 2e-2).
  2. main loop per 1024-node superblock (2 PSUM banks):
       - idx chunk DMA (2 KB) onto partition 0, ring of 8, Pool-queue trigger
       - broadcast idx across partitions via PE DoubleRow fp8 matmuls:
         idx = 16q + r split host-side, both fp8-exact; ones (x) [q; r]
         accumulates q+r = idx into PSUM at 0.5 cycles/column
       - DVE is_equal against a partition-iota column -> one-hot^T (fp16)
       - 8x PE DoubleRow matmuls: one-hot streams against the fp8 [hi; lo]
         table planes -> exact-hilo gather at 0.5 cycles/column
       - one PSUM -> SBUF staging copy (Scalar)
       - one 512 KiB DMA store per superblock

The only irreducible HBM traffic is the 25.7 MiB/core fp32 output write
(~75 us at ~350 GB/s); every other engine is scheduled to stay under that.
idx streams in per-superblock chunks because a single [1, N] row load
serializes ~38 us on one SBUF partition's write port, and engine APs cannot
address arbitrary base partitions (only 0/32/64) for a [49, 1024] layout.
"""

import sys

if "/opt/trn_rl_repo" not in sys.path:
    sys.path.insert(0, "/opt/trn_rl_repo")

import numpy as np

import concourse.bass as bass
import concourse.bacc as bacc
import concourse.tile as tile
from concourse import mybir
from concourse import bass_utils

F32 = mybir.dt.float32
F16 = mybir.dt.float16
F8 = mybir.dt.float8e4
DR = mybir.MatmulPerfMode.DoubleRow

N_NODES = 400000
H = 128          # hidden
G = 256          # graph_dim
B = 128          # batch (table rows)
N_CORES = 8
NSHARD = N_NODES // N_CORES          # 50000
SB = 1024                            # nodes per superblock (2 PSUM banks)
NPAD = 50176                         # 49 * 1024, per-core padded shard
NSB = NPAD // SB                     # 49
EPS = 1e-5

# Schedule knobs (variant tuple: (dr_bcast, dr_gather, copy_sched, pool_mod))
#   dr_bcast: broadcast via fp8 DoubleRow (else fp16 K=1 matmuls)
#   dr_gather: gather via fp8 hi+lo DoubleRow (else single fp16 matmul)
#   copy_sched: string over {"A","V"} cycled per superblock for the
#               PSUM->stage copy engine (Scalar/DVE; GpSimd cannot read PSUM)
#   pool_mod: m -> superblocks with sb % m == m-1 broadcast on GpSimd
#             partition_broadcast (None = all broadcasts on PE)
DEFAULT_VARIANT = (True, True, "A", None)


def _row1(ap):
    """View a 1-D DRAM AP as [1, N]."""
    return bass.AP(tensor=ap.tensor, offset=ap.offset, ap=[[0, 1]] + list(ap.ap))


def _bcast128(ap):
    """View a 1-D DRAM AP as [128, N] replicated across partitions."""
    return bass.AP(tensor=ap.tensor, offset=ap.offset, ap=[[0, 128]] + list(ap.ap))


def _dup2(ap):
    """Insert a stride-0 length-2 middle dim: [P, N] -> [P, 2, N] view."""
    a = list(ap.ap)
    assert len(a) == 2
    return bass.AP(tensor=ap.tensor, offset=ap.offset, ap=[a[0], [0, 2], a[1]])


def build_bass(variant=DEFAULT_VARIANT):
    dr_bcast, dr_gather, copy_sched, pool_mod = variant
    nc = bacc.Bacc("TRN2", target_bir_lowering=False)

    # host-transposed layouts (see _prep_in_maps)
    gvt_d = nc.dram_tensor("gvT", [G, B], F32, kind="ExternalInput")
    wvt_d = nc.dram_tensor("WvT", [G, H], F32, kind="ExternalInput")
    wivt_d = nc.dram_tensor("WivT", [H, H], F32, kind="ExternalInput")
    woutt_d = nc.dram_tensor("WoutT", [H, H], F32, kind="ExternalInput")
    bv_d = nc.dram_tensor("bv", [H], F32, kind="ExternalInput")
    biv_d = nc.dram_tensor("biv", [H], F32, kind="ExternalInput")
    bout_d = nc.dram_tensor("bout", [H], F32, kind="ExternalInput")
    gamma_d = nc.dram_tensor("gamma", [H], F32, kind="ExternalInput")
    beta_d = nc.dram_tensor("beta", [H], F32, kind="ExternalInput")
    # fp16 idx per superblock, and fp8 split layout [half][plane q|r][512]
    idx_d = nc.dram_tensor("idx", [NSB, SB], F16, kind="ExternalInput")
    idx8_d = nc.dram_tensor("idx8", [NSB, 2 * SB], F8, kind="ExternalInput")
    out_d = nc.dram_tensor("out", [NPAD, H], F32, kind="ExternalOutput")

    with tile.TileContext(nc) as tc:
        with (
            tc.tile_pool(name="singles", bufs=1) as singles,
            tc.tile_pool(name="oh", bufs=4) as oh_pool,
            tc.tile_pool(name="idxc", bufs=8) as idxc_pool,
            tc.tile_pool(name="bcsb", bufs=2) as bcsb_pool,
            tc.tile_pool(name="bps", bufs=2, space="PSUM") as bc_ps_pool,
            tc.tile_pool(name="ops", bufs=2, space="PSUM") as out_ps_pool,
            tc.tile_pool(name="stage", bufs=3) as stage_pool,
        ):
            # ---------- constants & weights ----------
            # gvT [256, 128] exceeds 128 partitions; load as two [128, 128]
            gvt0 = singles.tile([128, B], F32, tag="gvt0")
            nc.sync.dma_start(out=gvt0, in_=gvt_d[0:128, :])
            gvt1 = singles.tile([128, B], F32, tag="gvt1")
            nc.sync.dma_start(out=gvt1, in_=gvt_d[128:256, :])
            wvt0 = singles.tile([128, H], F32, tag="wvt0")
            nc.sync.dma_start(out=wvt0, in_=wvt_d[0:128, :])
            wvt1 = singles.tile([128, H], F32, tag="wvt1")
            nc.sync.dma_start(out=wvt1, in_=wvt_d[128:256, :])
            wivt_sb = singles.tile([H, H], F32, tag="wivt")
            nc.sync.dma_start(out=wivt_sb, in_=wivt_d[:, :])
            woutt_sb = singles.tile([H, H], F32, tag="woutt")
            nc.sync.dma_start(out=woutt_sb, in_=woutt_d[:, :])

            bv_sb = singles.tile([1, H], F32, tag="bv")
            nc.sync.dma_start(out=bv_sb, in_=_row1(bv_d[:]))
            biv_sb = singles.tile([1, H], F32, tag="biv")
            nc.sync.dma_start(out=biv_sb, in_=_row1(biv_d[:]))
            bout_sb = singles.tile([1, H], F32, tag="bout")
            nc.sync.dma_start(out=bout_sb, in_=_row1(bout_d[:]))

            gamma_gr = singles.tile([128, H], F32, tag="gamma_gr")
            nc.gpsimd.dma_start(out=gamma_gr, in_=_bcast128(gamma_d[:]))
            beta_gr = singles.tile([128, H], F32, tag="beta_gr")
            nc.gpsimd.dma_start(out=beta_gr, in_=_bcast128(beta_d[:]))

            ones32 = singles.tile([1, 128], F32, tag="ones32")
            nc.vector.memset(ones32, 1.0)
            ones16 = singles.tile([1, 128], F16, tag="ones16")
            nc.vector.memset(ones16, 1.0)
            ones8 = singles.tile([1, 2 * 128], F8, tag="ones8")
            nc.vector.memset(ones8, 1.0)
            eps_sb = singles.tile([128, 1], F32, tag="eps")
            nc.vector.memset(eps_sb, EPS)

            iota_i = singles.tile([128, 1], mybir.dt.int32, tag="iota_i")
            nc.gpsimd.iota(iota_i, [[0, 1]], base=0, channel_multiplier=1)
            iota_f = singles.tile([128, 1], F32, tag="iota_f")
            nc.vector.tensor_copy(out=iota_f, in_=iota_i)

            tc.strict_bb_all_engine_barrier()

            # ---------- table prologue ----------
            # PSUM borrowed from the out pool (same tag -> same 2 slots).
            # v.T[h, b] = sum_g WvT[g, h] * gvT[g, b] + bv[h]
            vt_ps = out_ps_pool.tile([128, SB], F32, tag="outps")
            nc.tensor.matmul(vt_ps[:, 0:128], wvt0, gvt0, start=True, stop=False)
            nc.tensor.matmul(vt_ps[:, 0:128], wvt1, gvt1, start=False, stop=False)
            nc.tensor.matmul(vt_ps[:, 0:128], bv_sb, ones32, start=False, stop=True)
            vt_sb = singles.tile([128, 128], F32, tag="vt_sb")
            nc.scalar.copy(out=vt_sb, in_=vt_ps[:, 0:128])

            # v2.T[j, b] = sum_h WivT[h, j] * vT[h, b] + biv[j]
            v2t_ps = out_ps_pool.tile([128, SB], F32, tag="outps")
            nc.tensor.matmul(v2t_ps[:, 0:128], wivt_sb, vt_sb, start=True, stop=False)
            nc.tensor.matmul(v2t_ps[:, 0:128], biv_sb, ones32, start=False, stop=True)
            v2t_sb = singles.tile([128, 128], F32, tag="v2t_sb")
            nc.scalar.copy(out=v2t_sb, in_=v2t_ps[:, 0:128])

            # ao[b, h2] = sum_j v2T[j, b] * WoutT[j, h2] + bout[h2]
            ao_ps = out_ps_pool.tile([128, SB], F32, tag="outps")
            nc.tensor.matmul(ao_ps[:, 0:128], v2t_sb, woutt_sb, start=True, stop=False)
            nc.tensor.matmul(ao_ps[:, 0:128], ones32, bout_sb, start=False, stop=True)

            # LayerNorm over free dim
            stats = singles.tile([128, 6], F32, tag="stats")
            nc.vector.bn_stats(out=stats, in_=ao_ps[:, 0:128])
            mv = singles.tile([128, 2], F32, tag="mv")
            nc.vector.bn_aggr(out=mv, in_=stats)
            rstd = singles.tile([128, 1], F32, tag="rstd")
            nc.scalar.activation(
                rstd, mv[:, 1:2], mybir.ActivationFunctionType.Sqrt,
                bias=eps_sb, scale=1.0,
            )
            nc.vector.reciprocal(out=rstd, in_=rstd)

            tbl = singles.tile([128, 128], F32, tag="tbl")
            nc.vector.tensor_scalar(
                out=tbl, in0=ao_ps[:, 0:128],
                scalar1=mv[:, 0:1], scalar2=rstd,
                op0=mybir.AluOpType.subtract, op1=mybir.AluOpType.mult,
            )
            tbl2 = singles.tile([128, 128], F32, tag="tbl2")
            nc.vector.tensor_mul(out=tbl2, in0=tbl, in1=gamma_gr)
            tbl3 = singles.tile([128, 128], F32, tag="tbl3")
            nc.vector.tensor_add(out=tbl3, in0=tbl2, in1=beta_gr)

            if dr_gather:
                # fp8 hi+lo planes side by side: [128, 0:128]=hi, [128,128:256]=lo
                tbl_hl = singles.tile([128, 2 * 128], F8, tag="tbl_hl")
                nc.vector.tensor_copy(out=tbl_hl[:, 0:128], in_=tbl3)
                hi32 = singles.tile([128, 128], F32, tag="hi32")
                nc.vector.tensor_copy(out=hi32, in_=tbl_hl[:, 0:128])
                resid = singles.tile([128, 128], F32, tag="resid")
                nc.vector.tensor_sub(out=resid, in0=tbl3, in1=hi32)
                nc.vector.tensor_copy(out=tbl_hl[:, 128:256], in_=resid)
                tbl_mv = tbl_hl.rearrange("p (j n) -> p j n", j=2)
            else:
                tbl_h = singles.tile([128, 128], F16, tag="tbl_h")
                nc.vector.tensor_copy(out=tbl_h, in_=tbl3)
                tbl_mv = tbl_h

            ones8_3d = ones8.rearrange("p (j n) -> p j n", j=2)

            # ---------- main gather loop: one superblock = 1024 nodes ----------
            for sb in range(NSB):
                use_pool = pool_mod is not None and sb % pool_mod == pool_mod - 1
                # idx chunk for this superblock: 2 KB onto partition 0,
                # triggered from the cheap Pool queue; ring depth 8 lets the
                # loads run ~7 superblocks ahead of the consumers.
                if use_pool or not dr_bcast:
                    idxc = idxc_pool.tile([1, SB], F16, tag="idxc16")
                    nc.gpsimd.dma_start(out=idxc, in_=idx_d[sb:sb + 1, :])
                else:
                    idxc = idxc_pool.tile([1, 2 * SB], F8, tag="idxc8")
                    nc.gpsimd.dma_start(out=idxc, in_=idx8_d[sb:sb + 1, :])
                if use_pool:
                    bc_sb = bcsb_pool.tile([128, SB], F16, tag="bcsb")
                    nc.gpsimd.partition_broadcast(bc_sb, idxc)
                    eq_in = bc_sb
                else:
                    bc_ps = bc_ps_pool.tile([128, SB], F32, tag="bcast")
                    for h in range(2):
                        osl = slice(h * 512, (h + 1) * 512)
                        if dr_bcast:
                            rhs = idxc[:, h * SB:(h + 1) * SB].rearrange(
                                "p (j n) -> p j n", j=2
                            )
                            nc.tensor.matmul(
                                bc_ps[:, osl], ones8_3d, rhs,
                                start=True, stop=True, perf_mode=DR,
                            )
                        else:
                            nc.tensor.matmul(
                                bc_ps[:, osl], ones16, idxc[:, osl],
                                start=True, stop=True,
                            )
                    eq_in = bc_ps
                # onehotT[j, p] = (idx[p] == j)
                oh = oh_pool.tile([128, SB], F16 if not dr_gather else F8, tag="oh")
                nc.vector.tensor_scalar(
                    out=oh, in0=eq_in,
                    scalar1=iota_f, scalar2=None,
                    op0=mybir.AluOpType.is_equal,
                )
                # out[p, h] = sum_j onehotT[j, p] * table[j, h]
                out_ps = out_ps_pool.tile([128, SB], F32, tag="outps")
                for t in range(SB // 128):
                    sl = slice(t * 128, (t + 1) * 128)
                    if dr_gather:
                        nc.tensor.matmul(
                            out_ps[:, sl], _dup2(oh[:, sl]), tbl_mv,
                            start=True, stop=True, perf_mode=DR,
                        )
                    else:
                        nc.tensor.matmul(
                            out_ps[:, sl], oh[:, sl], tbl_mv,
                            start=True, stop=True,
                        )
                stage = stage_pool.tile([128, SB], F32, tag="stage")
                if copy_sched[sb % len(copy_sched)] == "A":
                    nc.scalar.copy(out=stage, in_=out_ps)
                else:
                    nc.vector.tensor_copy(out=stage, in_=out_ps)
                # Node order is host-permuted so partition p owns DRAM rows
                # [p*NT, (p+1)*NT): every store descriptor is a contiguous
                # 8*512B run per partition (full DMA line rate).
                ts = SB // 128                        # 8 tiles per store
                dview = out_d[:, :].rearrange("(p t) c -> p t c", p=128)[
                    :, sb * ts:(sb + 1) * ts, :
                ]
                sview = stage.rearrange("p (t c) -> p t c", c=128)
                nc.sync.dma_start(out=dview, in_=sview)

    nc.finalize()
    return nc


_CACHE = {}


def _get_nc(variant=None):
    key = variant or DEFAULT_VARIANT
    if key not in _CACHE:
        _CACHE[key] = build_bass(variant=key)
    return _CACHE[key]


def _prep_in_maps(inputs):
    import ml_dtypes  # noqa: F401  (mybir.dt.np covers the fp8 type)

    f32c = lambda x: np.ascontiguousarray(np.asarray(x), dtype=np.float32)
    win = np.asarray(inputs["Win"], dtype=np.float32)
    bin_ = np.asarray(inputs["bin"], dtype=np.float32)
    shared = {
        "gvT": f32c(np.asarray(inputs["graph_vec"], dtype=np.float32).T),
        "WvT": f32c(np.asarray(inputs["Wv"], dtype=np.float32).T),
        "WivT": f32c(win[2 * H:3 * H, :].T),
        "WoutT": f32c(np.asarray(inputs["Wout"], dtype=np.float32).T),
        "bv": f32c(inputs["bv"]),
        "biv": f32c(bin_[2 * H:3 * H]),
        "bout": f32c(inputs["bout"]),
        "gamma": f32c(inputs["gamma"]),
        "beta": f32c(inputs["beta"]),
    }
    bi = np.asarray(inputs["batch_indices"]).astype(np.int64).reshape(N_CORES, NSHARD)
    idx_pad = np.zeros((N_CORES, NPAD), dtype=np.int64)
    idx_pad[:, :NSHARD] = bi
    # Permute so device tile t covers nodes {p*NT + t}: partition p then owns
    # the contiguous output-row block [p*NT, (p+1)*NT) (contiguous DMA runs).
    nt = NPAD // 128
    idx_tr = idx_pad.reshape(N_CORES, 128, nt).transpose(0, 2, 1)  # [c, t, p]
    idx_flat = idx_tr.reshape(N_CORES, NSB, SB)
    idx_f16 = idx_flat.astype(np.float16)  # exact: values < 2048
    # fp8 split: idx = q + r with q = 16*(idx//16), r = idx%16 (both exact)
    f8np = mybir.dt.np(F8)
    q = (idx_flat // 16 * 16).astype(f8np)
    r = (idx_flat % 16).astype(f8np)
    # layout per sb: [half h][plane q|r][512]
    qh = q.reshape(N_CORES, NSB, 2, 512)
    rh = r.reshape(N_CORES, NSB, 2, 512)
    idx8 = np.stack([qh, rh], axis=3)  # [c, NSB, half, plane, 512]
    idx8 = idx8.reshape(N_CORES, NSB, 2 * SB)
    return [
        {
            **shared,
            "idx": np.ascontiguousarray(idx_f16[c]),
            "idx8": np.ascontiguousarray(idx8[c]),
        }
        for c in range(N_CORES)
    ]


def run_sharded(inputs, trace=False, variant=None, **kwargs):
    """Run the SPMD bass kernel on 8 cores; returns (output, BassKernelResults)."""
    kwargs.pop("precision", None)  # legacy knob
    in_maps = _prep_in_maps(inputs)
    nc = _get_nc(variant)
    res = bass_utils.run_bass_kernel_spmd(
        nc, in_maps, core_ids=list(range(N_CORES)), trace=trace, **kwargs
    )
    shards = [r["out"][:NSHARD] for r in res.results]
    out = np.concatenate(shards, axis=0)
    return out, res


def kernel(**inputs) -> np.ndarray:
    out, _ = run_sharded(inputs)
    return out
